# revision 10
# baseline (speedup 1.0000x reference)
"""MoE layer (8 experts, top-2) on 8 Trainium2 NeuronCores — expert parallel.

Strategy
--------
Host (inside kernel(), cheap O(T*D) work):
  * gate: logits = x @ Wg, softmax, top-2, normalized combine weights
  * dispatch: gather each expert's tokens, pad to a common capacity CAP
  * combine: out[t] += w * (y + b2[e]) scatter-add

Device (one expert per core, SPMD over 8 cores, >99% of FLOPs):
  * yT = W2[e].T @ gelu(W1[e].T @ xT + b1[e])  with all matmuls on TensorE
  * weights resident in SBUF (bf16), fp32 PSUM accumulation
  * layout keeps D/F on partitions and tokens on the matmul free dim, so
    no transposes are needed anywhere on device

Returns the full [B, S, D] float32 output.
"""

import os
import sys

for _p in ("/opt/trn_rl_repo",):
    if _p not in sys.path:
        sys.path.insert(0, _p)

import numpy as np
import ml_dtypes

import concourse.bass as bass
import concourse.mybir as mybir
import concourse.tile as tile
from concourse import bacc
from concourse.bass_utils import run_bass_kernel_spmd

D_MODEL = 1024
D_FF = 4096
NUM_EXPERTS = 8
TOP_K = 2
N_CORES = 8
P = 128  # SBUF partitions

DC = D_MODEL // P   # 8 chunks of the model dim
FC = D_FF // P      # 32 chunks of the ffn dim

LAST_EXEC_NS = None

_program_cache = {}


def _install_profile_hook():
    """Provide antenv.axon_hooks (NTFF profiling) if the image lacks it."""
    import types
    import contextlib
    import ctypes
    try:
        from antenv.axon_hooks import get_axon_ntff_profile_hook  # noqa: F401
        return
    except ImportError:
        pass
    so = "/opt/axon/libaxon_pjrt.so"
    if not os.path.exists(so):
        return
    lib = ctypes.CDLL(so)
    if not hasattr(lib, "axon_start_nrt_profile"):
        return
    lib.axon_start_nrt_profile.argtypes = [ctypes.POINTER(ctypes.c_int64),
                                           ctypes.c_size_t]
    lib.axon_start_nrt_profile.restype = ctypes.c_int64
    lib.axon_stop_nrt_profile.argtypes = [ctypes.c_char_p]
    lib.axon_stop_nrt_profile.restype = ctypes.c_int64

    @contextlib.contextmanager
    def _hook(output_dir, device_ids):
        import jax
        jax.devices()
        if device_ids:
            ids = (ctypes.c_int64 * len(device_ids))(*device_ids)
            rc = lib.axon_start_nrt_profile(ids, len(device_ids))
        else:
            rc = lib.axon_start_nrt_profile(None, 0)
        if rc != 0:
            raise RuntimeError(f"axon_start_nrt_profile rc={rc}")
        try:
            yield
        finally:
            n = lib.axon_stop_nrt_profile(str(output_dir).encode())
            print(f"profile: {n} ntff file(s) -> {output_dir}", file=sys.stderr)

    mod = types.ModuleType("antenv.axon_hooks")
    mod.get_axon_ntff_profile_hook = lambda: _hook
    mod.set_axon_ntff_profile_hook = lambda h: None
    sys.modules["antenv.axon_hooks"] = mod
    import antenv
    antenv.axon_hooks = mod
    import concourse.bass_utils as _bu
    _bu.upload_artifacts = lambda tmpdir: tmpdir


def _token_tiles(cap):
    """Split CAP tokens into matmul free-dim tiles (<=512 each)."""
    tiles = []
    t0 = 0
    while t0 < cap:
        tn = min(512, cap - t0)
        tiles.append((t0, tn))
        t0 += tn
    return tiles


def _build_program(cap):
    """SPMD program: one expert's FFN over [cap] tokens, bf16 matmuls."""
    bf16 = mybir.dt.bfloat16
    f32 = mybir.dt.float32
    nc = bacc.Bacc("TRN2", target_bir_lowering=False, debug=False,
                   num_devices=N_CORES)

    xT_d = nc.dram_tensor("xT", [D_MODEL, cap], bf16, kind="ExternalInput").ap()
    w1_d = nc.dram_tensor("W1", [D_MODEL, D_FF], bf16, kind="ExternalInput").ap()
    w2_d = nc.dram_tensor("W2", [D_FF, D_MODEL], bf16, kind="ExternalInput").ap()
    b1_d = nc.dram_tensor("b1", [P, FC], f32, kind="ExternalInput").ap()
    yT_d = nc.dram_tensor("yT", [D_MODEL, cap], f32, kind="ExternalOutput").ap()

    w1_r = w1_d.rearrange("(dc p) f -> p dc f", p=P)     # [128, 8, 4096]
    w2_r = w2_d.rearrange("(fc p) d -> p fc d", p=P)     # [128, 32, 1024]
    xT_r = xT_d.rearrange("(dc p) t -> p dc t", p=P)     # [128, 8, cap]

    NQ = 4  # weight load split for DMA/compute overlap
    FQ = D_FF // NQ

    with tile.TileContext(nc) as tc:
        with (
            tc.tile_pool(name="wpool", bufs=1) as wpool,
            tc.tile_pool(name="hpool", bufs=1) as hpool,
            tc.tile_pool(name="ypool", bufs=4) as ypool,
            tc.tile_pool(name="ph", bufs=2, space="PSUM") as ph_pool,
            tc.tile_pool(name="py", bufs=2, space="PSUM") as py_pool,
        ):
            xs = wpool.tile([P, DC, cap], bf16)
            nc.sync.dma_start(xs[:], xT_r)
            b1s = wpool.tile([P, FC], f32)
            nc.sync.dma_start(b1s[:], b1_d)

            # W1 split into 4 fc-range quarters so mm1 can start early
            w1q = []
            for q in range(NQ):
                wq = wpool.tile([P, DC, FQ], bf16, tag=f"w1q{q}")
                nc.sync.dma_start(wq[:], w1_r[:, :, q * FQ:(q + 1) * FQ])
                w1q.append(wq)
            w2q = []
            fc_per_q = FC // NQ
            for q in range(NQ):
                wq = wpool.tile([P, fc_per_q, D_MODEL], bf16, tag=f"w2q{q}")
                nc.sync.dma_start(wq[:], w2_r[:, q * fc_per_q:(q + 1) * fc_per_q, :])
                w2q.append(wq)

            for (t0, tn) in _token_tiles(cap):
                # hT = gelu(W1.T @ x + b1), layout [F(part), tokens]
                hT = hpool.tile([P, FC, 512], bf16, tag="hT")
                for fc in range(FC):
                    ph = ph_pool.tile([P, tn], f32, tag="ph")
                    q, fi = divmod(fc, fc_per_q)
                    for dc in range(DC):
                        nc.tensor.matmul(
                            ph[:],
                            w1q[q][:, dc, fi * P:(fi + 1) * P],
                            xs[:, dc, t0:t0 + tn],
                            start=(dc == 0),
                            stop=(dc == DC - 1),
                        )
                    nc.scalar.activation(
                        hT[:, fc, :tn], ph[:],
                        mybir.ActivationFunctionType.Gelu,
                        bias=b1s[:, fc:fc + 1], scale=1.0,
                    )

                # yT = W2.T @ hT, layout [D(part), tokens]
                for dc in range(DC):
                    py = py_pool.tile([P, tn], f32, tag="py")
                    for fc in range(FC):
                        q, fi = divmod(fc, fc_per_q)
                        nc.tensor.matmul(
                            py[:],
                            w2q[q][:, fi, dc * P:(dc + 1) * P],
                            hT[:, fc, :tn],
                            start=(fc == 0),
                            stop=(fc == FC - 1),
                        )
                    yt = ypool.tile([P, tn], f32, tag="yt")
                    nc.vector.tensor_copy(yt[:], py[:])
                    nc.sync.dma_start(yT_d[dc * P:(dc + 1) * P, t0:t0 + tn], yt[:])

    nc.compile()
    return nc


def _route(x_flat, Wg):
    """Replicate the reference gate in float64: softmax, top-2, renorm."""
    logits = x_flat.astype(np.float64) @ Wg.astype(np.float64)
    logits -= logits.max(axis=-1, keepdims=True)
    p = np.exp(logits)
    p /= p.sum(axis=-1, keepdims=True)
    order = np.argsort(-p, axis=-1, kind="stable")[:, :TOP_K]   # [T, 2]
    rows = np.arange(p.shape[0])[:, None]
    tv = p[rows, order]                                          # [T, 2]
    tvn = tv / (tv.sum(axis=-1, keepdims=True) + 1e-8)
    return order, tvn


def kernel(x, Wg, W1, b1, W2, b2):
    global LAST_EXEC_NS
    x = np.asarray(x, dtype=np.float32)
    Wg = np.asarray(Wg, dtype=np.float32)
    W1 = np.asarray(W1, dtype=np.float32)
    b1 = np.asarray(b1, dtype=np.float32)
    W2 = np.asarray(W2, dtype=np.float32)
    b2 = np.asarray(b2, dtype=np.float32)

    B, S, D = x.shape
    x_flat = x.reshape(-1, D)
    T = x_flat.shape[0]

    order, tvn = _route(x_flat, Wg)

    idx = []
    wts = []
    for e in range(NUM_EXPERTS):
        sel = np.nonzero((order == e).any(axis=1))[0]
        idx.append(sel)
        wmat = np.where(order[sel] == e, tvn[sel], 0.0)
        wts.append(wmat.sum(axis=-1))                            # [cnt]

    max_cnt = max(len(s) for s in idx)
    cap = max(P, ((max_cnt + P - 1) // P) * P)

    nc = _program_cache.get(cap)
    if nc is None:
        nc = _build_program(cap)
        _program_cache[cap] = nc

    bf16 = ml_dtypes.bfloat16
    in_maps = []
    for e in range(NUM_EXPERTS):
        sel = idx[e]
        xT = np.zeros((D_MODEL, cap), dtype=bf16)
        xT[:, :len(sel)] = x_flat[sel].T.astype(bf16)
        in_maps.append({
            "xT": xT,
            "W1": W1[e].astype(bf16),
            "W2": W2[e].astype(bf16),
            "b1": np.ascontiguousarray(b1[e].reshape(FC, P).T),
        })

    trace = bool(os.environ.get("MOE_TRACE"))
    if trace:
        _install_profile_hook()
    res = run_bass_kernel_spmd(
        nc, in_maps, list(range(N_CORES)),
        trace=trace,
        tmpdir=os.environ.get("MOE_TRACE_DIR") or None,
    )
    LAST_EXEC_NS = res.exec_time_ns

    out = np.zeros((T, D_MODEL), dtype=np.float64)
    for e in range(NUM_EXPERTS):
        sel = idx[e]
        yT = np.asarray(res.results[e]["yT"])                    # [D, cap] f32
        y = yT[:, :len(sel)].T.astype(np.float64)
        out[sel] += wts[e][:, None] * (y + b2[e].astype(np.float64))

    return out.reshape(B, S, D_MODEL).astype(np.float32)
